# revision 23
# baseline (speedup 1.0000x reference)
"""GQA (H=32, KV=8, D=128, T=2048, hid=4096) causal attention + RoPE,
tensor-parallel over heads across 8 NeuronCores.

Sharding: core i owns kv-head i and query heads 4i..4i+3.

Fully interleaved pipeline, one group per 512-token chunk tcn:
    qkv-projection(tcn) -> attention(all 4 heads, q-chunk tcn)
    -> o_proj(chunk tcn-1) -> AllGather(chunk tcn)
so the per-chunk AllGather flies under the next chunk's compute and the
PE never waits on a phase boundary. The AllGather payload is the chunk's
attention output [512, 512] (4 heads x 128 d), gathered to [4096, 512]
in original head-major row order, so wo needs no permutation.

Details:
  - All matmul operands bf16 (1 cyc/row on the PE at any free size, half
    the DMA/SBUF of fp32r); PSUM accumulation fp32.
  - RoPE fused into the projection epilogue (partition-half swap via
    SBUF-SBUF DMA), K epilogue emitted first so attention unblocks early.
  - Causal attention in S_T [kt, qt] layout, unnormalized softmax
    (scores are +-9, exp fp32-safe), denominator via ones-vector matmul
    accumulated in PSUM, then: Act copy -> K=1 ones-row matmul broadcast
    -> DVE fast approx reciprocal -> scale. Diagonal tiles for qc>=1 are
    column-trimmed (emitted after the full-width tiles so PSUM
    start/stop flags land on full-width writes).
  - PSUM is six [128,512] fp32 banks shared by tag aliasing across the
    qkv accumulators, V-transpose, attention S/O/broadcast tiles and the
    o_proj accumulators, plus a double-buffered [1,512] denominator.
Host concatenates the 8 column slices of o_proj output.
"""

import math
import numpy as np
import ml_dtypes

import concourse.bass as bass
import concourse.mybir as mybir
import concourse.tile as tile
from concourse import bacc
from concourse.bass_utils import run_bass_kernel_spmd

T = 2048
HID = 4096
H = 32
KV = 8
D = 128
NC = 8
HQ = H // NC          # 4 query heads per core
DQ = HQ * D           # 512
KT = HID // 128       # 32 contraction tiles
TC = T // 512         # 4 t-chunks
ROPE_BASE = 10000.0

MM_DT = mybir.dt.bfloat16
R32 = mybir.dt.float32r
F32 = mybir.dt.float32
EXP = mybir.ActivationFunctionType.Exp
MUL = mybir.AluOpType.mult
ADD = mybir.AluOpType.add

_BUILD_CACHE = {}
RUN_KWARGS = {}  # test harness hook (e.g. {"trace": True})


def _build_nc():
    nc = bacc.Bacc(None, target_bir_lowering=False, num_devices=NC)

    xT = nc.declare_dram_parameter("xT", [HID, T], MM_DT, isOutput=False)
    wq = nc.declare_dram_parameter("wq", [HID, DQ], MM_DT, isOutput=False)
    wk = nc.declare_dram_parameter("wk", [HID, D], MM_DT, isOutput=False)
    wv = nc.declare_dram_parameter("wv", [HID, D], MM_DT, isOutput=False)
    wo = nc.declare_dram_parameter("wo", [HID, DQ], MM_DT, isOutput=False)
    cosT = nc.declare_dram_parameter("cosT", [D, T], MM_DT, isOutput=False)
    sinT = nc.declare_dram_parameter("sinT", [D, T], MM_DT, isOutput=False)  # sign-folded
    masks = nc.declare_dram_parameter("masks", [128, 4 * 512], MM_DT, isOutput=False)
    ones = nc.declare_dram_parameter("ones", [128, 1], MM_DT, isOutput=False)
    onesr = nc.declare_dram_parameter("onesr", [1, 128], R32, isOutput=False)
    ident = nc.declare_dram_parameter("ident", [128, 128], F32, isOutput=False)
    out = nc.declare_dram_parameter("out", [T, DQ], F32, isOutput=True)

    # per-chunk attention output: [4 heads x 128 d, 512 t] -> gathered
    # [8 cores x 512, 512] with rows in original (core, head, d) order
    attT_loc = [nc.dram_tensor(f"attT_loc{c}", [DQ, 512], MM_DT)
                for c in range(TC)]
    attT_full = [nc.dram_tensor(f"attT_full{c}", [HID, 512], MM_DT,
                                addr_space="Shared") for c in range(TC)]

    inv_sqrt_d = 1.0 / math.sqrt(D)

    with tile.TileContext(nc) as tc:
        with (
            tc.tile_pool(name="persist", bufs=1) as pp,
            tc.tile_pool(name="mm", bufs=1, space="PSUM") as mm,
            tc.tile_pool(name="denp", bufs=2, space="PSUM") as denp,
            tc.tile_pool(name="xrhs", bufs=7) as xp,
            tc.tile_pool(name="ropetmp", bufs=1) as rp,
            tc.tile_pool(name="attn", bufs=3) as ap,
            tc.tile_pool(name="attout", bufs=2) as aop,
            tc.tile_pool(name="ostrip", bufs=2) as osp,
            tc.tile_pool(name="oout", bufs=2) as oop,
        ):
            # ---- persistent SBUF ----
            qt_sb = [[pp.tile([128, 512], MM_DT, tag=f"qt{h}_{c}",
                              name=f"qt{h}_{c}") for c in range(TC)]
                     for h in range(HQ)]
            kt_sb = [pp.tile([128, 512], MM_DT, tag=f"kt_{c}", name=f"kt_{c}")
                     for c in range(TC)]
            vt_sb = [pp.tile([128, 512], F32, tag=f"vt_{c}", name=f"vt_{c}")
                     for c in range(TC)]
            vn_sb = [pp.tile([128, 512], MM_DT, tag=f"vn_{c}", name=f"vn_{c}")
                     for c in range(TC)]
            cos_sb = pp.tile([128, T], MM_DT, tag="cos")
            sin_sb = pp.tile([128, T], MM_DT, tag="sin")
            msk_sb = pp.tile([128, 2048], MM_DT, tag="msk")
            ones_sb = pp.tile([128, 1], MM_DT, tag="ones")
            onesr_sb = pp.tile([1, 128], R32, tag="onesr")
            id_sb = pp.tile([128, 128], F32, tag="ident")
            wq_sb = pp.tile([128, KT * DQ], MM_DT, tag="wq")
            wk_sb = pp.tile([128, KT * D], MM_DT, tag="wk")
            wv_sb = pp.tile([128, KT * D], MM_DT, tag="wv")
            wo_sb = pp.tile([128, KT * DQ], MM_DT, tag="wo")

            # ---- input DMAs: k/v weights first (first matmuls), then q ----
            nc.sync.dma_start(
                wk_sb[:, :].rearrange("p (a m) -> p a m", a=KT),
                wk.rearrange("(a p) m -> p a m", p=128))
            nc.sync.dma_start(
                wv_sb[:, :].rearrange("p (a m) -> p a m", a=KT),
                wv.rearrange("(a p) m -> p a m", p=128))
            # consts on the gpsimd queue, off the wk/wv/wq/x path
            nc.gpsimd.dma_start(cos_sb[:, :], cosT[:, :])
            nc.gpsimd.dma_start(sin_sb[:, :], sinT[:, :])
            nc.gpsimd.dma_start(msk_sb[:, :], masks[:, :])
            nc.gpsimd.dma_start(ones_sb[:, :], ones[:, :])
            nc.gpsimd.dma_start(onesr_sb[:, :], onesr[:, :])
            nc.gpsimd.dma_start(id_sb[:, :], ident[:, :])

            # mm-pool tag plan (all [128,512] F32, 6 banks):
            #   qkv:        pq0-3 -> A B C D, pk -> E, pv -> F
            #   V transp:   F
            #   attention:  s_ps cycles A B C, bc_ps D, o_ps alternates E F
            #   o_proj:     accumulators alternate A B
            def mmt(tag):
                return mm.tile([128, 512], F32, tag=tag, name=f"mm_{tag}")

            xT_r = xT.rearrange("(a p) t -> p a t", p=128)
            xt_pending = {}

            def issue_xt(tcn, ka):
                t = xp.tile([128, 4 * 512], MM_DT, tag="xt", name="xt")
                nc.sync.dma_start(
                    t[:, :].rearrange("p (a f) -> p a f", a=4),
                    xT_r[:, 4 * ka:4 * (ka + 1), tcn * 512:(tcn + 1) * 512])
                xt_pending[(tcn, ka)] = t

            def get_xt(tcn, ka):
                if (tcn, ka) not in xt_pending:
                    issue_xt(tcn, ka)
                return xt_pending.pop((tcn, ka))

            issue_xt(0, 0)
            nc.sync.dma_start(
                wq_sb[:, :].rearrange("p (a m) -> p a m", a=KT),
                wq.rearrange("(a p) m -> p a m", p=128))

            for tcn in range(TC):
                _qkv_chunk(nc, tcn, mmt, get_xt, rp, wq_sb, wk_sb, wv_sb,
                           qt_sb, kt_sb, vt_sb, vn_sb, cos_sb, sin_sb, id_sb)
                if tcn + 1 < TC:  # hide DMA-queue contention at the boundary
                    for ka in range(4):
                        issue_xt(tcn + 1, ka)
                for h in range(HQ):
                    _attn_tile(nc, h, tcn, mmt, denp, ap, aop,
                               qt_sb, kt_sb, vn_sb, msk_sb, ones_sb,
                               onesr_sb, attT_loc, inv_sqrt_d)
                    if h == 1 and tcn in (1, 2):
                        _oproj_chunk(nc, tcn - 1, mmt, osp, oop,
                                     wo_sb, attT_full, out)
                nc.gpsimd.collective_compute(
                    "AllGather",
                    mybir.AluOpType.bypass,
                    replica_groups=[list(range(NC))],
                    ins=[attT_loc[tcn][:, :]],
                    outs=[attT_full[tcn][:, :]],
                )
                if tcn == 0:
                    nc.gpsimd.dma_start(
                        wo_sb[:, :].rearrange("p (a m) -> p a m", a=KT),
                        wo.rearrange("(a p) m -> p a m", p=128))
            _oproj_chunk(nc, TC - 2, mmt, osp, oop, wo_sb, attT_full, out)
            _oproj_chunk(nc, TC - 1, mmt, osp, oop, wo_sb, attT_full, out)

    nc.compile()
    return nc


def _qkv_chunk(nc, tcn, mmt, get_xt, rp, wq_sb, wk_sb, wv_sb,
               qt_sb, kt_sb, vt_sb, vn_sb, cos_sb, sin_sb, id_sb):
    ts = tcn * 512
    pq = [mmt("ABCD"[h]) for h in range(HQ)]
    pk = mmt("E")
    pv = mmt("F")
    for ka in range(KT // 4):
        xt4 = get_xt(tcn, ka)
        for j in range(4):
            k = 4 * ka + j
            xt = xt4[:, j * 512:(j + 1) * 512]
            nc.tensor.matmul(
                pk[:, :], wk_sb[:, k * D:(k + 1) * D], xt,
                start=(k == 0), stop=(k == KT - 1), skip_group_check=True)
            nc.tensor.matmul(
                pv[:, :], wv_sb[:, k * D:(k + 1) * D], xt,
                start=(k == 0), stop=(k == KT - 1), skip_group_check=True)
            for h in range(HQ):
                nc.tensor.matmul(
                    pq[h][:, :],
                    wq_sb[:, k * DQ + h * 128: k * DQ + (h + 1) * 128],
                    xt,
                    start=(k == 0), stop=(k == KT - 1), skip_group_check=True)

    # RoPE epilogue: K first (unblocks attention S), then q heads
    for g in range(HQ + 1):
        src = pk if g == 0 else pq[g - 1]
        dst = kt_sb[tcn] if g == 0 else qt_sb[g - 1][tcn]
        qn_t = rp.tile([128, 512], F32, tag="qnat")
        nc.scalar.copy(qn_t[:, :], src[:, :])
        sh_t = rp.tile([128, 512], F32, tag="qshuf")
        nc.scalar.dma_start(sh_t[0:64, :], qn_t[64:128, :])
        nc.scalar.dma_start(sh_t[64:128, :], qn_t[0:64, :])
        qc_t = rp.tile([128, 512], F32, tag="qcos")
        nc.vector.tensor_tensor(
            qc_t[:, :], src[:, :], cos_sb[:, ts:ts + 512], op=MUL)
        ss_t = rp.tile([128, 512], F32, tag="qsin")
        nc.vector.tensor_tensor(
            ss_t[:, :], sh_t[:, :], sin_sb[:, ts:ts + 512], op=MUL)
        nc.vector.tensor_tensor(dst[:, :], qc_t[:, :], ss_t[:, :], op=ADD)
    nc.scalar.copy(vt_sb[tcn][:, :], pv[:, :])

    # V transpose: 4x [128,128] into the F-tag PSUM bank, one copy out
    vp = mmt("F")
    for i in range(4):
        nc.tensor.transpose(
            vp[:, i * 128:(i + 1) * 128],
            vt_sb[tcn][:, i * 128:(i + 1) * 128], id_sb[:, :])
    nc.scalar.copy(vn_sb[tcn][:, :], vp[:, :])


def _attn_tile(nc, h, qc, mmt, denp, ap, aop, qt_sb, kt_sb, vn_sb,
               msk_sb, ones_sb, onesr_sb, attT_loc, inv_sqrt_d):
    """Attention for (head h, q-chunk qc): S^T = K^T-stationary scores,
    unnormalized softmax, PV accumulation, then normalize.

    kt order: full-width tiles first (off-diagonals ascending, then the
    m=0 diagonal LAST so PSUM start/stop land on full-width writes);
    trimmed diagonal tiles m=1..3 in between (qc>=1 only)."""
    qs_tile = qt_sb[h][qc]
    o_ps = mmt("EF"[h % 2])
    den_ps = denp.tile([1, 512], F32, tag="den")
    if qc == 0:
        # qc=0 has no full-width off-diagonal tiles: keep full width and
        # mask each diagonal class m after exp (start/stop stay full)
        order = [(kt, 0) for kt in range(4)]
    else:
        n_kt = 4 * (qc + 1)
        order = [(kt, 0) for kt in range(4 * qc)]          # off-diag full
        order += [(4 * qc + m, 128 * m) for m in (1, 2, 3)]  # trimmed diag
        order += [(4 * qc, 0)]                               # m=0 last
    last = len(order) - 1
    for i, (kt, cs) in enumerate(order):
        m = kt - 4 * qc
        s_ps = mmt("ABC"[i % 3])
        nc.tensor.matmul(
            s_ps[:, cs:512],
            kt_sb[kt // 4][:, (kt % 4) * 128:(kt % 4 + 1) * 128],
            qs_tile[:, cs:512],
            start=True, stop=True, skip_group_check=True)
        e_t = ap.tile([128, 512], MM_DT, tag="et")
        if m >= 0 and qc == 0:
            # full width, mask class m after exp
            e_raw = ap.tile([128, 512], F32, tag="eraw")
            nc.scalar.activation(e_raw[:, :], s_ps[:, :], EXP,
                                 scale=inv_sqrt_d)
            nc.vector.tensor_tensor(
                e_t[:, :], e_raw[:, :], msk_sb[:, m * 512:(m + 1) * 512],
                op=MUL)
        elif m >= 0:
            # trimmed diagonal: triangle block [cs, cs+128) + full tail
            e_raw = ap.tile([128, 128], F32, tag="erawn")
            nc.scalar.activation(e_raw[:, :], s_ps[:, cs:cs + 128], EXP,
                                 scale=inv_sqrt_d)
            nc.vector.tensor_tensor(
                e_t[:, cs:cs + 128], e_raw[:, :], msk_sb[:, 0:128], op=MUL)
            if cs + 128 < 512:
                nc.scalar.activation(
                    e_t[:, cs + 128:512], s_ps[:, cs + 128:512], EXP,
                    scale=inv_sqrt_d)
        else:
            nc.scalar.activation(e_t[:, :], s_ps[:, :], EXP,
                                 scale=inv_sqrt_d)
        nc.tensor.matmul(
            den_ps[:, cs:512], ones_sb[:, :], e_t[:, cs:512],
            start=(i == 0), stop=(i == last), skip_group_check=True)
        nc.tensor.matmul(
            o_ps[:, cs:512],
            vn_sb[kt // 4][:, (kt % 4) * 128:(kt % 4 + 1) * 128],
            e_t[:, cs:512],
            start=(i == 0), stop=(i == last), skip_group_check=True)
    # normalize: broadcast den via K=1 matmul, approx-reciprocal, scale
    den_sb = aop.tile([1, 512], R32, tag="densb")
    nc.scalar.copy(den_sb[:, :], den_ps[:, :])
    bc_ps = mmt("D")
    nc.tensor.matmul(
        bc_ps[:, :], onesr_sb[:, :], den_sb[:, :],
        start=True, stop=True, skip_group_check=True)
    rcp_t = aop.tile([128, 512], F32, tag="rcp")
    nc.vector.reciprocal_approx_fast(rcp_t[:, :], bc_ps[:, :])
    at_t = aop.tile([128, 512], MM_DT, tag="attT")
    nc.vector.tensor_tensor(at_t[:, :], o_ps[:, :], rcp_t[:, :], op=MUL)
    nc.gpsimd.dma_start(attT_loc[qc][h * 128:(h + 1) * 128, :], at_t[:, :])


def _oproj_chunk(nc, qc, mmt, osp, oop, wo_sb, attT_full, out):
    att_r = attT_full[qc].rearrange("(a p) t -> p a t", p=128)
    for f in range(4):
        tt = qc * 4 + f
        strip = osp.tile([128, KT * 128], MM_DT, tag="strip")
        nc.scalar.dma_start(
            strip[:, :].rearrange("p (a f) -> p a f", a=KT),
            att_r[:, :, f * 128:(f + 1) * 128])
        o_ps = mmt("AB"[f % 2])
        for k2 in range(KT):
            nc.tensor.matmul(
                o_ps[:, :],
                strip[:, k2 * 128:(k2 + 1) * 128],
                wo_sb[:, k2 * DQ:(k2 + 1) * DQ],
                start=(k2 == 0), stop=(k2 == KT - 1), skip_group_check=True)
        ot = oop.tile([128, 512], F32, tag="ot")
        nc.scalar.copy(ot[:, :], o_ps[:, :])
        nc.gpsimd.dma_start(out[tt * 128:(tt + 1) * 128, :], ot[:, :])


def _host_consts():
    # rope tables, transposed + sign-folded
    inv = 1.0 / (ROPE_BASE ** (np.arange(0, D, 2, dtype=np.float32) / D))
    t = np.arange(T, dtype=np.float32)
    f = np.outer(t, inv)
    e = np.concatenate([f, f], axis=-1)
    cos = np.cos(e).astype(np.float32)
    sin = np.sin(e).astype(np.float32)
    sgn = np.where(np.arange(D) < D // 2, -1.0, 1.0).astype(np.float32)
    cosT = np.ascontiguousarray(cos.T)
    sinT = np.ascontiguousarray((sin * sgn).T)
    # causal 0/1 masks for the 4 diagonal kt-tile classes: keep iff f - p >= 128*m
    p = np.arange(128)[:, None]
    fr = np.arange(512)[None, :]
    msk = np.concatenate(
        [(fr - p >= 128 * m).astype(np.float32) for m in range(4)], axis=1)
    ones = np.ones((128, 1), np.float32)
    onesr = np.ones((1, 128), np.float32)
    ident = np.eye(128, dtype=np.float32)
    return cosT, sinT, msk, ones, onesr, ident


def kernel(x, wq, wk, wv, wo, mask=None, **_ignored):
    BF16 = ml_dtypes.bfloat16
    x = np.asarray(x, dtype=np.float32)
    wq = np.asarray(wq, dtype=np.float32)
    wk = np.asarray(wk, dtype=np.float32)
    wv = np.asarray(wv, dtype=np.float32)
    wo = np.asarray(wo, dtype=np.float32)
    B = x.shape[0]
    xT = np.ascontiguousarray(x.reshape(T, HID).T).astype(BF16)   # [HID, T]
    cosT, sinT, msk, ones, onesr, ident = _host_consts()

    if "nc" not in _BUILD_CACHE:
        _BUILD_CACHE["nc"] = _build_nc()
    nc = _BUILD_CACHE["nc"]

    in_maps = []
    for i in range(NC):
        in_maps.append({
            "xT": xT,
            "wq": np.ascontiguousarray(wq[:, i * DQ:(i + 1) * DQ]).astype(BF16),
            "wk": np.ascontiguousarray(wk[:, i * D:(i + 1) * D]).astype(BF16),
            "wv": np.ascontiguousarray(wv[:, i * D:(i + 1) * D]).astype(BF16),
            "wo": np.ascontiguousarray(wo[:, i * DQ:(i + 1) * DQ]).astype(BF16),
            "cosT": cosT.astype(BF16), "sinT": sinT.astype(BF16),
            "masks": msk.astype(BF16),
            "ones": ones.astype(BF16), "onesr": onesr, "ident": ident,
        })

    res = run_bass_kernel_spmd(nc, in_maps, core_ids=list(range(NC)), **RUN_KWARGS)
    _BUILD_CACHE["last_res"] = res
    out = np.concatenate([res.results[i]["out"] for i in range(NC)], axis=1)
    return out.reshape(B, T, HID)


if __name__ == "__main__":
    rng = np.random.default_rng(0)
    s = 1.0 / math.sqrt(HID)
    x = rng.standard_normal((1, T, HID), dtype=np.float32)
    wq_ = rng.standard_normal((HID, H * D), dtype=np.float32) * s
    wk_ = rng.standard_normal((HID, KV * D), dtype=np.float32) * s
    wv_ = rng.standard_normal((HID, KV * D), dtype=np.float32) * s
    wo_ = rng.standard_normal((H * D, HID), dtype=np.float32) * s
    o = kernel(x, wq_, wk_, wv_, wo_, None)
    print("out", o.shape, o.dtype, float(np.abs(o).mean()))


# revision 24
# speedup vs baseline: 1.0043x; 1.0043x over previous
"""GQA (H=32, KV=8, D=128, T=2048, hid=4096) causal attention + RoPE,
tensor-parallel over heads across 8 NeuronCores.

Sharding: core i owns kv-head i and query heads 4i..4i+3.

Fully interleaved pipeline, one group per 512-token chunk tcn:
    qkv-projection(tcn) -> attention(all 4 heads, q-chunk tcn)
    -> o_proj(chunk tcn-1) -> AllGather(chunk tcn)
so the per-chunk AllGather flies under the next chunk's compute and the
PE never waits on a phase boundary. The AllGather payload is the chunk's
attention output [512, 512] (4 heads x 128 d), gathered to [4096, 512]
in original head-major row order, so wo needs no permutation.

Details:
  - All matmul operands bf16 (1 cyc/row on the PE at any free size, half
    the DMA/SBUF of fp32r); PSUM accumulation fp32.
  - RoPE fused into the projection epilogue (partition-half swap via
    SBUF-SBUF DMA), K epilogue emitted first so attention unblocks early.
  - Causal attention in S_T [kt, qt] layout, unnormalized softmax
    (scores are +-9, exp fp32-safe), denominator via ones-vector matmul
    accumulated in PSUM, then: Act copy -> K=1 ones-row matmul broadcast
    -> DVE fast approx reciprocal -> scale. Diagonal tiles for qc>=1 are
    column-trimmed (emitted after the full-width tiles so PSUM
    start/stop flags land on full-width writes).
  - PSUM is six [128,512] fp32 banks shared by tag aliasing across the
    qkv accumulators, V-transpose, attention S/O/broadcast tiles and the
    o_proj accumulators, plus a double-buffered [1,512] denominator.
Host concatenates the 8 column slices of o_proj output.
"""

import math
import numpy as np
import ml_dtypes

import concourse.bass as bass
import concourse.mybir as mybir
import concourse.tile as tile
from concourse import bacc
from concourse.bass_utils import run_bass_kernel_spmd

T = 2048
HID = 4096
H = 32
KV = 8
D = 128
NC = 8
HQ = H // NC          # 4 query heads per core
DQ = HQ * D           # 512
KT = HID // 128       # 32 contraction tiles
TC = T // 512         # 4 t-chunks
ROPE_BASE = 10000.0

MM_DT = mybir.dt.bfloat16
R32 = mybir.dt.float32r
F32 = mybir.dt.float32
EXP = mybir.ActivationFunctionType.Exp
MUL = mybir.AluOpType.mult
ADD = mybir.AluOpType.add

_BUILD_CACHE = {}
RUN_KWARGS = {}  # test harness hook (e.g. {"trace": True})


def _build_nc():
    nc = bacc.Bacc(None, target_bir_lowering=False, num_devices=NC)

    xT = nc.declare_dram_parameter("xT", [HID, T], MM_DT, isOutput=False)
    wq = nc.declare_dram_parameter("wq", [HID, DQ], MM_DT, isOutput=False)
    wk = nc.declare_dram_parameter("wk", [HID, D], MM_DT, isOutput=False)
    wv = nc.declare_dram_parameter("wv", [HID, D], MM_DT, isOutput=False)
    wo = nc.declare_dram_parameter("wo", [HID, DQ], MM_DT, isOutput=False)
    cosT = nc.declare_dram_parameter("cosT", [D, T], MM_DT, isOutput=False)
    sinT = nc.declare_dram_parameter("sinT", [D, T], MM_DT, isOutput=False)  # sign-folded
    masks = nc.declare_dram_parameter("masks", [128, 4 * 512], MM_DT, isOutput=False)
    ones = nc.declare_dram_parameter("ones", [128, 1], MM_DT, isOutput=False)
    onesr = nc.declare_dram_parameter("onesr", [1, 128], R32, isOutput=False)
    ident = nc.declare_dram_parameter("ident", [128, 128], F32, isOutput=False)
    out = nc.declare_dram_parameter("out", [T, DQ], F32, isOutput=True)

    # per-chunk attention output: [4 heads x 128 d, 512 t] -> gathered
    # [8 cores x 512, 512] with rows in original (core, head, d) order
    attT_loc = [nc.dram_tensor(f"attT_loc{c}", [DQ, 512], MM_DT)
                for c in range(TC)]
    attT_full = [nc.dram_tensor(f"attT_full{c}", [HID, 512], MM_DT,
                                addr_space="Shared") for c in range(TC)]

    inv_sqrt_d = 1.0 / math.sqrt(D)

    with tile.TileContext(nc) as tc:
        with (
            tc.tile_pool(name="persist", bufs=1) as pp,
            tc.tile_pool(name="mm", bufs=1, space="PSUM") as mm,
            tc.tile_pool(name="denp", bufs=2, space="PSUM") as denp,
            tc.tile_pool(name="xrhs", bufs=8) as xp,
            tc.tile_pool(name="ropetmp", bufs=1) as rp,
            tc.tile_pool(name="attn", bufs=3) as ap,
            tc.tile_pool(name="attout", bufs=2) as aop,
            tc.tile_pool(name="ostrip", bufs=2) as osp,
            tc.tile_pool(name="oout", bufs=1) as oop,
        ):
            # ---- persistent SBUF ----
            qt_sb = [[pp.tile([128, 512], MM_DT, tag=f"qt{h}_{c}",
                              name=f"qt{h}_{c}") for c in range(TC)]
                     for h in range(HQ)]
            kt_sb = [pp.tile([128, 512], MM_DT, tag=f"kt_{c}", name=f"kt_{c}")
                     for c in range(TC)]
            vt_sb = [pp.tile([128, 512], F32, tag=f"vt_{c}", name=f"vt_{c}")
                     for c in range(TC)]
            vn_sb = [pp.tile([128, 512], MM_DT, tag=f"vn_{c}", name=f"vn_{c}")
                     for c in range(TC)]
            cos_sb = pp.tile([128, T], MM_DT, tag="cos")
            sin_sb = pp.tile([128, T], MM_DT, tag="sin")
            msk_sb = pp.tile([128, 2048], MM_DT, tag="msk")
            ones_sb = pp.tile([128, 1], MM_DT, tag="ones")
            onesr_sb = pp.tile([1, 128], R32, tag="onesr")
            id_sb = pp.tile([128, 128], F32, tag="ident")
            wq_sb = pp.tile([128, KT * DQ], MM_DT, tag="wq")
            wk_sb = pp.tile([128, KT * D], MM_DT, tag="wk")
            wv_sb = pp.tile([128, KT * D], MM_DT, tag="wv")
            wo_sb = pp.tile([128, KT * DQ], MM_DT, tag="wo")

            # ---- input DMAs: k/v weights first (first matmuls), then q ----
            nc.sync.dma_start(
                wk_sb[:, :].rearrange("p (a m) -> p a m", a=KT),
                wk.rearrange("(a p) m -> p a m", p=128))
            nc.sync.dma_start(
                wv_sb[:, :].rearrange("p (a m) -> p a m", a=KT),
                wv.rearrange("(a p) m -> p a m", p=128))
            # consts on the gpsimd queue, off the wk/wv/wq/x path
            nc.gpsimd.dma_start(cos_sb[:, :], cosT[:, :])
            nc.gpsimd.dma_start(sin_sb[:, :], sinT[:, :])
            nc.gpsimd.dma_start(msk_sb[:, :], masks[:, :])
            nc.gpsimd.dma_start(ones_sb[:, :], ones[:, :])
            nc.gpsimd.dma_start(onesr_sb[:, :], onesr[:, :])
            nc.gpsimd.dma_start(id_sb[:, :], ident[:, :])

            # mm-pool tag plan (all [128,512] F32, 6 banks):
            #   qkv:        pq0-3 -> A B C D, pk -> E, pv -> F
            #   V transp:   F
            #   attention:  s_ps cycles A B C, bc_ps D, o_ps alternates E F
            #   o_proj:     accumulators alternate A B
            def mmt(tag):
                return mm.tile([128, 512], F32, tag=tag, name=f"mm_{tag}")

            xT_r = xT.rearrange("(a p) t -> p a t", p=128)
            xt_pending = {}

            def issue_xt(tcn, ka):
                t = xp.tile([128, 4 * 512], MM_DT, tag="xt", name="xt")
                nc.sync.dma_start(
                    t[:, :].rearrange("p (a f) -> p a f", a=4),
                    xT_r[:, 4 * ka:4 * (ka + 1), tcn * 512:(tcn + 1) * 512])
                xt_pending[(tcn, ka)] = t

            def get_xt(tcn, ka):
                if (tcn, ka) not in xt_pending:
                    issue_xt(tcn, ka)
                return xt_pending.pop((tcn, ka))

            issue_xt(0, 0)
            nc.sync.dma_start(
                wq_sb[:, :].rearrange("p (a m) -> p a m", a=KT),
                wq.rearrange("(a p) m -> p a m", p=128))

            for tcn in range(TC):
                _qkv_chunk(nc, tcn, mmt, get_xt, rp, wq_sb, wk_sb, wv_sb,
                           qt_sb, kt_sb, vt_sb, vn_sb, cos_sb, sin_sb, id_sb)
                if tcn + 1 < TC:  # hide DMA-queue contention at the boundary
                    for ka in range(6):
                        issue_xt(tcn + 1, ka)
                for h in range(HQ):
                    _attn_tile(nc, h, tcn, mmt, denp, ap, aop,
                               qt_sb, kt_sb, vn_sb, msk_sb, ones_sb,
                               onesr_sb, attT_loc, inv_sqrt_d)
                    if h == 1 and tcn in (1, 2):
                        _oproj_chunk(nc, tcn - 1, mmt, osp, oop,
                                     wo_sb, attT_full, out)
                nc.gpsimd.collective_compute(
                    "AllGather",
                    mybir.AluOpType.bypass,
                    replica_groups=[list(range(NC))],
                    ins=[attT_loc[tcn][:, :]],
                    outs=[attT_full[tcn][:, :]],
                )
                if tcn == 0:
                    nc.gpsimd.dma_start(
                        wo_sb[:, :].rearrange("p (a m) -> p a m", a=KT),
                        wo.rearrange("(a p) m -> p a m", p=128))
            _oproj_chunk(nc, TC - 2, mmt, osp, oop, wo_sb, attT_full, out)
            _oproj_chunk(nc, TC - 1, mmt, osp, oop, wo_sb, attT_full, out)

    nc.compile()
    return nc


def _qkv_chunk(nc, tcn, mmt, get_xt, rp, wq_sb, wk_sb, wv_sb,
               qt_sb, kt_sb, vt_sb, vn_sb, cos_sb, sin_sb, id_sb):
    ts = tcn * 512
    pq = [mmt("ABCD"[h]) for h in range(HQ)]
    pk = mmt("E")
    pv = mmt("F")
    for ka in range(KT // 4):
        xt4 = get_xt(tcn, ka)
        for j in range(4):
            k = 4 * ka + j
            xt = xt4[:, j * 512:(j + 1) * 512]
            nc.tensor.matmul(
                pk[:, :], wk_sb[:, k * D:(k + 1) * D], xt,
                start=(k == 0), stop=(k == KT - 1), skip_group_check=True)
            nc.tensor.matmul(
                pv[:, :], wv_sb[:, k * D:(k + 1) * D], xt,
                start=(k == 0), stop=(k == KT - 1), skip_group_check=True)
            for h in range(HQ):
                nc.tensor.matmul(
                    pq[h][:, :],
                    wq_sb[:, k * DQ + h * 128: k * DQ + (h + 1) * 128],
                    xt,
                    start=(k == 0), stop=(k == KT - 1), skip_group_check=True)

    # RoPE epilogue: K first (unblocks attention S), then q heads
    for g in range(HQ + 1):
        src = pk if g == 0 else pq[g - 1]
        dst = kt_sb[tcn] if g == 0 else qt_sb[g - 1][tcn]
        qn_t = rp.tile([128, 512], F32, tag="qnat")
        nc.scalar.copy(qn_t[:, :], src[:, :])
        sh_t = rp.tile([128, 512], F32, tag="qshuf")
        nc.scalar.dma_start(sh_t[0:64, :], qn_t[64:128, :])
        nc.scalar.dma_start(sh_t[64:128, :], qn_t[0:64, :])
        qc_t = rp.tile([128, 512], F32, tag="qcos")
        nc.vector.tensor_tensor(
            qc_t[:, :], src[:, :], cos_sb[:, ts:ts + 512], op=MUL)
        ss_t = rp.tile([128, 512], F32, tag="qsin")
        nc.vector.tensor_tensor(
            ss_t[:, :], sh_t[:, :], sin_sb[:, ts:ts + 512], op=MUL)
        nc.vector.tensor_tensor(dst[:, :], qc_t[:, :], ss_t[:, :], op=ADD)
    nc.scalar.copy(vt_sb[tcn][:, :], pv[:, :])

    # V transpose: 4x [128,128] into the F-tag PSUM bank, one copy out
    vp = mmt("F")
    for i in range(4):
        nc.tensor.transpose(
            vp[:, i * 128:(i + 1) * 128],
            vt_sb[tcn][:, i * 128:(i + 1) * 128], id_sb[:, :])
    nc.scalar.copy(vn_sb[tcn][:, :], vp[:, :])


def _attn_tile(nc, h, qc, mmt, denp, ap, aop, qt_sb, kt_sb, vn_sb,
               msk_sb, ones_sb, onesr_sb, attT_loc, inv_sqrt_d):
    """Attention for (head h, q-chunk qc): S^T = K^T-stationary scores,
    unnormalized softmax, PV accumulation, then normalize.

    kt order: full-width tiles first (off-diagonals ascending, then the
    m=0 diagonal LAST so PSUM start/stop land on full-width writes);
    trimmed diagonal tiles m=1..3 in between (qc>=1 only)."""
    qs_tile = qt_sb[h][qc]
    o_ps = mmt("EF"[h % 2])
    den_ps = denp.tile([1, 512], F32, tag="den")
    if qc == 0:
        # qc=0 has no full-width off-diagonal tiles: keep full width and
        # mask each diagonal class m after exp (start/stop stay full)
        order = [(kt, 0) for kt in range(4)]
    else:
        n_kt = 4 * (qc + 1)
        order = [(kt, 0) for kt in range(4 * qc)]          # off-diag full
        order += [(4 * qc + m, 128 * m) for m in (1, 2, 3)]  # trimmed diag
        order += [(4 * qc, 0)]                               # m=0 last
    last = len(order) - 1
    for i, (kt, cs) in enumerate(order):
        m = kt - 4 * qc
        s_ps = mmt("ABC"[i % 3])
        nc.tensor.matmul(
            s_ps[:, cs:512],
            kt_sb[kt // 4][:, (kt % 4) * 128:(kt % 4 + 1) * 128],
            qs_tile[:, cs:512],
            start=True, stop=True, skip_group_check=True)
        e_t = ap.tile([128, 512], MM_DT, tag="et")
        if m >= 0 and qc == 0:
            # full width, mask class m after exp
            e_raw = ap.tile([128, 512], F32, tag="eraw")
            nc.scalar.activation(e_raw[:, :], s_ps[:, :], EXP,
                                 scale=inv_sqrt_d)
            nc.vector.tensor_tensor(
                e_t[:, :], e_raw[:, :], msk_sb[:, m * 512:(m + 1) * 512],
                op=MUL)
        elif m >= 0:
            # trimmed diagonal: triangle block [cs, cs+128) + full tail
            e_raw = ap.tile([128, 128], F32, tag="erawn")
            nc.scalar.activation(e_raw[:, :], s_ps[:, cs:cs + 128], EXP,
                                 scale=inv_sqrt_d)
            nc.vector.tensor_tensor(
                e_t[:, cs:cs + 128], e_raw[:, :], msk_sb[:, 0:128], op=MUL)
            if cs + 128 < 512:
                nc.scalar.activation(
                    e_t[:, cs + 128:512], s_ps[:, cs + 128:512], EXP,
                    scale=inv_sqrt_d)
        else:
            nc.scalar.activation(e_t[:, :], s_ps[:, :], EXP,
                                 scale=inv_sqrt_d)
        nc.tensor.matmul(
            den_ps[:, cs:512], ones_sb[:, :], e_t[:, cs:512],
            start=(i == 0), stop=(i == last), skip_group_check=True)
        nc.tensor.matmul(
            o_ps[:, cs:512],
            vn_sb[kt // 4][:, (kt % 4) * 128:(kt % 4 + 1) * 128],
            e_t[:, cs:512],
            start=(i == 0), stop=(i == last), skip_group_check=True)
    # normalize: broadcast den via K=1 matmul, approx-reciprocal, scale
    den_sb = aop.tile([1, 512], R32, tag="densb")
    nc.scalar.copy(den_sb[:, :], den_ps[:, :])
    bc_ps = mmt("D")
    nc.tensor.matmul(
        bc_ps[:, :], onesr_sb[:, :], den_sb[:, :],
        start=True, stop=True, skip_group_check=True)
    rcp_t = aop.tile([128, 512], F32, tag="rcp")
    nc.vector.reciprocal_approx_fast(rcp_t[:, :], bc_ps[:, :])
    at_t = aop.tile([128, 512], MM_DT, tag="attT")
    nc.vector.tensor_tensor(at_t[:, :], o_ps[:, :], rcp_t[:, :], op=MUL)
    nc.gpsimd.dma_start(attT_loc[qc][h * 128:(h + 1) * 128, :], at_t[:, :])


def _oproj_chunk(nc, qc, mmt, osp, oop, wo_sb, attT_full, out):
    att_r = attT_full[qc].rearrange("(a p) t -> p a t", p=128)
    for f in range(4):
        tt = qc * 4 + f
        strip = osp.tile([128, KT * 128], MM_DT, tag="strip")
        nc.gpsimd.dma_start(
            strip[:, :].rearrange("p (a f) -> p a f", a=KT),
            att_r[:, :, f * 128:(f + 1) * 128])
        o_ps = mmt("AB"[f % 2])
        for k2 in range(KT):
            nc.tensor.matmul(
                o_ps[:, :],
                strip[:, k2 * 128:(k2 + 1) * 128],
                wo_sb[:, k2 * DQ:(k2 + 1) * DQ],
                start=(k2 == 0), stop=(k2 == KT - 1), skip_group_check=True)
        ot = oop.tile([128, 512], F32, tag="ot")
        nc.scalar.copy(ot[:, :], o_ps[:, :])
        nc.gpsimd.dma_start(out[tt * 128:(tt + 1) * 128, :], ot[:, :])


def _host_consts():
    # rope tables, transposed + sign-folded
    inv = 1.0 / (ROPE_BASE ** (np.arange(0, D, 2, dtype=np.float32) / D))
    t = np.arange(T, dtype=np.float32)
    f = np.outer(t, inv)
    e = np.concatenate([f, f], axis=-1)
    cos = np.cos(e).astype(np.float32)
    sin = np.sin(e).astype(np.float32)
    sgn = np.where(np.arange(D) < D // 2, -1.0, 1.0).astype(np.float32)
    cosT = np.ascontiguousarray(cos.T)
    sinT = np.ascontiguousarray((sin * sgn).T)
    # causal 0/1 masks for the 4 diagonal kt-tile classes: keep iff f - p >= 128*m
    p = np.arange(128)[:, None]
    fr = np.arange(512)[None, :]
    msk = np.concatenate(
        [(fr - p >= 128 * m).astype(np.float32) for m in range(4)], axis=1)
    ones = np.ones((128, 1), np.float32)
    onesr = np.ones((1, 128), np.float32)
    ident = np.eye(128, dtype=np.float32)
    return cosT, sinT, msk, ones, onesr, ident


def kernel(x, wq, wk, wv, wo, mask=None, **_ignored):
    BF16 = ml_dtypes.bfloat16
    x = np.asarray(x, dtype=np.float32)
    wq = np.asarray(wq, dtype=np.float32)
    wk = np.asarray(wk, dtype=np.float32)
    wv = np.asarray(wv, dtype=np.float32)
    wo = np.asarray(wo, dtype=np.float32)
    B = x.shape[0]
    xT = np.ascontiguousarray(x.reshape(T, HID).T).astype(BF16)   # [HID, T]
    cosT, sinT, msk, ones, onesr, ident = _host_consts()

    if "nc" not in _BUILD_CACHE:
        _BUILD_CACHE["nc"] = _build_nc()
    nc = _BUILD_CACHE["nc"]

    in_maps = []
    for i in range(NC):
        in_maps.append({
            "xT": xT,
            "wq": np.ascontiguousarray(wq[:, i * DQ:(i + 1) * DQ]).astype(BF16),
            "wk": np.ascontiguousarray(wk[:, i * D:(i + 1) * D]).astype(BF16),
            "wv": np.ascontiguousarray(wv[:, i * D:(i + 1) * D]).astype(BF16),
            "wo": np.ascontiguousarray(wo[:, i * DQ:(i + 1) * DQ]).astype(BF16),
            "cosT": cosT.astype(BF16), "sinT": sinT.astype(BF16),
            "masks": msk.astype(BF16),
            "ones": ones.astype(BF16), "onesr": onesr, "ident": ident,
        })

    res = run_bass_kernel_spmd(nc, in_maps, core_ids=list(range(NC)), **RUN_KWARGS)
    _BUILD_CACHE["last_res"] = res
    out = np.concatenate([res.results[i]["out"] for i in range(NC)], axis=1)
    return out.reshape(B, T, HID)


if __name__ == "__main__":
    rng = np.random.default_rng(0)
    s = 1.0 / math.sqrt(HID)
    x = rng.standard_normal((1, T, HID), dtype=np.float32)
    wq_ = rng.standard_normal((HID, H * D), dtype=np.float32) * s
    wk_ = rng.standard_normal((HID, KV * D), dtype=np.float32) * s
    wv_ = rng.standard_normal((HID, KV * D), dtype=np.float32) * s
    wo_ = rng.standard_normal((H * D, HID), dtype=np.float32) * s
    o = kernel(x, wq_, wk_, wv_, wo_, None)
    print("out", o.shape, o.dtype, float(np.abs(o).mean()))


# revision 25
# speedup vs baseline: 1.1061x; 1.1013x over previous
"""GQA (H=32, KV=8, D=128, T=2048, hid=4096) causal attention + RoPE,
tensor-parallel over heads across 8 NeuronCores.

Sharding: core i owns kv-head i and query heads 4i..4i+3.

Fully interleaved pipeline, one group per 512-token chunk tcn:
    qkv-projection(tcn) -> attention(all 4 heads, q-chunk tcn)
    -> o_proj(chunk tcn-1) -> AllGather(chunk tcn)
so the per-chunk AllGather flies under the next chunk's compute and the
PE never waits on a phase boundary. The AllGather payload is the chunk's
attention output [512, 512] (4 heads x 128 d), gathered to [4096, 512]
in original head-major row order, so wo needs no permutation.

Details:
  - All matmul operands bf16 (1 cyc/row on the PE at any free size, half
    the DMA/SBUF of fp32r); PSUM accumulation fp32.
  - RoPE fused into the projection epilogue (partition-half swap via
    SBUF-SBUF DMA), K epilogue emitted first so attention unblocks early.
  - Causal attention in S_T [kt, qt] layout, unnormalized softmax
    (scores are +-9, exp fp32-safe), denominator via ones-vector matmul
    accumulated in PSUM, then: Act copy -> K=1 ones-row matmul broadcast
    -> DVE fast approx reciprocal -> scale. Diagonal tiles for qc>=1 are
    column-trimmed (emitted after the full-width tiles so PSUM
    start/stop flags land on full-width writes).
  - PSUM is six [128,512] fp32 banks shared by tag aliasing across the
    qkv accumulators, V-transpose, attention S/O/broadcast tiles and the
    o_proj accumulators, plus a double-buffered [1,512] denominator.
Host concatenates the 8 column slices of o_proj output.
"""

import math
import numpy as np
import ml_dtypes

import concourse.bass as bass
import concourse.mybir as mybir
import concourse.tile as tile
from concourse import bacc
from concourse.bass_utils import run_bass_kernel_spmd

T = 2048
HID = 4096
H = 32
KV = 8
D = 128
NC = 8
HQ = H // NC          # 4 query heads per core
DQ = HQ * D           # 512
KT = HID // 128       # 32 contraction tiles
TC = T // 512         # 4 t-chunks
ROPE_BASE = 10000.0

MM_DT = mybir.dt.bfloat16
R32 = mybir.dt.float32r
F32 = mybir.dt.float32
EXP = mybir.ActivationFunctionType.Exp
MUL = mybir.AluOpType.mult
ADD = mybir.AluOpType.add

_BUILD_CACHE = {}
RUN_KWARGS = {}  # test harness hook (e.g. {"trace": True})


def _build_nc():
    nc = bacc.Bacc(None, target_bir_lowering=False, num_devices=NC)

    xT = nc.declare_dram_parameter("xT", [HID, T], MM_DT, isOutput=False)
    wq = nc.declare_dram_parameter("wq", [HID, DQ], MM_DT, isOutput=False)
    wk = nc.declare_dram_parameter("wk", [HID, D], MM_DT, isOutput=False)
    wv = nc.declare_dram_parameter("wv", [HID, D], MM_DT, isOutput=False)
    wo = nc.declare_dram_parameter("wo", [HID, DQ], MM_DT, isOutput=False)
    cosT = nc.declare_dram_parameter("cosT", [D, T], MM_DT, isOutput=False)
    sinT = nc.declare_dram_parameter("sinT", [D, T], MM_DT, isOutput=False)  # sign-folded
    masks = nc.declare_dram_parameter("masks", [128, 4 * 512], MM_DT, isOutput=False)
    ones = nc.declare_dram_parameter("ones", [128, 1], MM_DT, isOutput=False)
    onesr = nc.declare_dram_parameter("onesr", [1, 128], R32, isOutput=False)
    ident = nc.declare_dram_parameter("ident", [128, 128], F32, isOutput=False)
    out = nc.declare_dram_parameter("out", [T, DQ], F32, isOutput=True)

    # per-chunk attention output: [4 heads x 128 d, 512 t] -> gathered
    # [8 cores x 512, 512] with rows in original (core, head, d) order
    attT_loc = [nc.dram_tensor(f"attT_loc{c}", [DQ, 512], MM_DT)
                for c in range(TC)]
    attT_full = [nc.dram_tensor(f"attT_full{c}", [HID, 512], MM_DT,
                                addr_space="Shared") for c in range(TC)]

    inv_sqrt_d = 1.0 / math.sqrt(D)

    with tile.TileContext(nc) as tc:
        with (
            tc.tile_pool(name="persist", bufs=1) as pp,
            tc.tile_pool(name="mm", bufs=1, space="PSUM") as mm,
            tc.tile_pool(name="denp", bufs=2, space="PSUM") as denp,
            tc.tile_pool(name="xrhs", bufs=8) as xp,
            tc.tile_pool(name="ropetmp", bufs=1) as rp,
            tc.tile_pool(name="attn", bufs=3) as ap,
            tc.tile_pool(name="attout", bufs=2) as aop,
            tc.tile_pool(name="ostrip", bufs=2) as osp,
            tc.tile_pool(name="oout", bufs=1) as oop,
        ):
            # ---- persistent SBUF ----
            qt_sb = [[pp.tile([128, 512], MM_DT, tag=f"qt{h}_{c}",
                              name=f"qt{h}_{c}") for c in range(TC)]
                     for h in range(HQ)]
            kt_sb = [pp.tile([128, 512], MM_DT, tag=f"kt_{c}", name=f"kt_{c}")
                     for c in range(TC)]
            vt_sb = [pp.tile([128, 512], F32, tag=f"vt_{c}", name=f"vt_{c}")
                     for c in range(TC)]
            vn_sb = [pp.tile([128, 512], MM_DT, tag=f"vn_{c}", name=f"vn_{c}")
                     for c in range(TC)]
            cos_sb = pp.tile([128, T], MM_DT, tag="cos")
            sin_sb = pp.tile([128, T], MM_DT, tag="sin")
            msk_sb = pp.tile([128, 2048], MM_DT, tag="msk")
            ones_sb = pp.tile([128, 1], MM_DT, tag="ones")
            onesr_sb = pp.tile([1, 128], R32, tag="onesr")
            id_sb = pp.tile([128, 128], F32, tag="ident")
            wq_sb = pp.tile([128, KT * DQ], MM_DT, tag="wq")
            wk_sb = pp.tile([128, KT * D], MM_DT, tag="wk")
            wv_sb = pp.tile([128, KT * D], MM_DT, tag="wv")
            wo_sb = pp.tile([128, KT * DQ], MM_DT, tag="wo")

            # ---- input DMAs: k/v weights first (first matmuls), then q ----
            nc.sync.dma_start(
                wk_sb[:, :].rearrange("p (a m) -> p a m", a=KT),
                wk.rearrange("(a p) m -> p a m", p=128))
            nc.sync.dma_start(
                wv_sb[:, :].rearrange("p (a m) -> p a m", a=KT),
                wv.rearrange("(a p) m -> p a m", p=128))
            # consts on the gpsimd queue, off the wk/wv/wq/x path
            nc.gpsimd.dma_start(cos_sb[:, :], cosT[:, :])
            nc.gpsimd.dma_start(sin_sb[:, :], sinT[:, :])
            nc.gpsimd.dma_start(msk_sb[:, :], masks[:, :])
            nc.gpsimd.dma_start(ones_sb[:, :], ones[:, :])
            nc.gpsimd.dma_start(onesr_sb[:, :], onesr[:, :])
            nc.gpsimd.dma_start(id_sb[:, :], ident[:, :])

            # mm-pool tag plan (all [128,512] F32, 6 banks):
            #   qkv:        pq0-3 -> A B C D, pk -> E, pv -> F
            #   V transp:   F
            #   attention:  s_ps cycles A B C, bc_ps D, o_ps alternates E F
            #   o_proj:     accumulators alternate A B
            def mmt(tag):
                return mm.tile([128, 512], F32, tag=tag, name=f"mm_{tag}")

            xT_r = xT.rearrange("(a p) t -> p a t", p=128)
            xt_pending = {}

            def issue_xt(tcn, ka):
                t = xp.tile([128, 4 * 512], MM_DT, tag="xt", name="xt")
                nc.sync.dma_start(
                    t[:, :].rearrange("p (a f) -> p a f", a=4),
                    xT_r[:, 4 * ka:4 * (ka + 1), tcn * 512:(tcn + 1) * 512])
                xt_pending[(tcn, ka)] = t

            def get_xt(tcn, ka):
                if (tcn, ka) not in xt_pending:
                    issue_xt(tcn, ka)
                return xt_pending.pop((tcn, ka))

            issue_xt(0, 0)
            nc.sync.dma_start(
                wq_sb[:, :].rearrange("p (a m) -> p a m", a=KT),
                wq.rearrange("(a p) m -> p a m", p=128))

            # phase 1: all qkv chunks (xt/weight DMA traffic stays alone)
            for tcn in range(TC):
                _qkv_chunk(nc, tcn, mmt, get_xt, rp, wq_sb, wk_sb, wv_sb,
                           qt_sb, kt_sb, vt_sb, vn_sb, cos_sb, sin_sb, id_sb)
                if tcn + 1 < TC:
                    for ka in range(4):
                        issue_xt(tcn + 1, ka)
                if tcn == 0:
                    nc.gpsimd.dma_start(
                        wo_sb[:, :].rearrange("p (a m) -> p a m", a=KT),
                        wo.rearrange("(a p) m -> p a m", p=128))
            # phase 2: attention qc-major; per-chunk AllGather flies under
            # the next q-chunk's attention
            for qc in range(TC):
                for h in range(HQ):
                    _attn_tile(nc, h, qc, mmt, denp, ap, aop,
                               qt_sb, kt_sb, vn_sb, msk_sb, ones_sb,
                               onesr_sb, attT_loc, inv_sqrt_d)
                nc.gpsimd.collective_compute(
                    "AllGather",
                    mybir.AluOpType.bypass,
                    replica_groups=[list(range(NC))],
                    ins=[attT_loc[qc][:, :]],
                    outs=[attT_full[qc][:, :]],
                )
            # phase 3: o_proj; only chunk 3's gather can still be in flight
            for qc in range(TC):
                _oproj_chunk(nc, qc, mmt, osp, oop, wo_sb, attT_full, out)

    nc.compile()
    return nc


def _qkv_chunk(nc, tcn, mmt, get_xt, rp, wq_sb, wk_sb, wv_sb,
               qt_sb, kt_sb, vt_sb, vn_sb, cos_sb, sin_sb, id_sb):
    ts = tcn * 512
    pq = [mmt("ABCD"[h]) for h in range(HQ)]
    pk = mmt("E")
    pv = mmt("F")
    for ka in range(KT // 4):
        xt4 = get_xt(tcn, ka)
        for j in range(4):
            k = 4 * ka + j
            xt = xt4[:, j * 512:(j + 1) * 512]
            nc.tensor.matmul(
                pk[:, :], wk_sb[:, k * D:(k + 1) * D], xt,
                start=(k == 0), stop=(k == KT - 1), skip_group_check=True)
            nc.tensor.matmul(
                pv[:, :], wv_sb[:, k * D:(k + 1) * D], xt,
                start=(k == 0), stop=(k == KT - 1), skip_group_check=True)
            for h in range(HQ):
                nc.tensor.matmul(
                    pq[h][:, :],
                    wq_sb[:, k * DQ + h * 128: k * DQ + (h + 1) * 128],
                    xt,
                    start=(k == 0), stop=(k == KT - 1), skip_group_check=True)

    # RoPE epilogue: K first (unblocks attention S), then q heads
    for g in range(HQ + 1):
        src = pk if g == 0 else pq[g - 1]
        dst = kt_sb[tcn] if g == 0 else qt_sb[g - 1][tcn]
        qn_t = rp.tile([128, 512], F32, tag="qnat")
        nc.scalar.copy(qn_t[:, :], src[:, :])
        sh_t = rp.tile([128, 512], F32, tag="qshuf")
        nc.scalar.dma_start(sh_t[0:64, :], qn_t[64:128, :])
        nc.scalar.dma_start(sh_t[64:128, :], qn_t[0:64, :])
        qc_t = rp.tile([128, 512], F32, tag="qcos")
        nc.vector.tensor_tensor(
            qc_t[:, :], src[:, :], cos_sb[:, ts:ts + 512], op=MUL)
        ss_t = rp.tile([128, 512], F32, tag="qsin")
        nc.vector.tensor_tensor(
            ss_t[:, :], sh_t[:, :], sin_sb[:, ts:ts + 512], op=MUL)
        nc.vector.tensor_tensor(dst[:, :], qc_t[:, :], ss_t[:, :], op=ADD)
    nc.scalar.copy(vt_sb[tcn][:, :], pv[:, :])

    # V transpose: 4x [128,128] into the F-tag PSUM bank, one copy out
    vp = mmt("F")
    for i in range(4):
        nc.tensor.transpose(
            vp[:, i * 128:(i + 1) * 128],
            vt_sb[tcn][:, i * 128:(i + 1) * 128], id_sb[:, :])
    nc.scalar.copy(vn_sb[tcn][:, :], vp[:, :])


def _attn_tile(nc, h, qc, mmt, denp, ap, aop, qt_sb, kt_sb, vn_sb,
               msk_sb, ones_sb, onesr_sb, attT_loc, inv_sqrt_d):
    """Attention for (head h, q-chunk qc): S^T = K^T-stationary scores,
    unnormalized softmax, PV accumulation, then normalize.

    kt order: full-width tiles first (off-diagonals ascending, then the
    m=0 diagonal LAST so PSUM start/stop land on full-width writes);
    trimmed diagonal tiles m=1..3 in between (qc>=1 only)."""
    qs_tile = qt_sb[h][qc]
    o_ps = mmt("EF"[h % 2])
    den_ps = denp.tile([1, 512], F32, tag="den")
    if qc == 0:
        # qc=0 has no full-width off-diagonal tiles: keep full width and
        # mask each diagonal class m after exp (start/stop stay full)
        order = [(kt, 0) for kt in range(4)]
    else:
        n_kt = 4 * (qc + 1)
        order = [(kt, 0) for kt in range(4 * qc)]          # off-diag full
        order += [(4 * qc + m, 128 * m) for m in (1, 2, 3)]  # trimmed diag
        order += [(4 * qc, 0)]                               # m=0 last
    last = len(order) - 1
    for i, (kt, cs) in enumerate(order):
        m = kt - 4 * qc
        s_ps = mmt("ABC"[i % 3])
        nc.tensor.matmul(
            s_ps[:, cs:512],
            kt_sb[kt // 4][:, (kt % 4) * 128:(kt % 4 + 1) * 128],
            qs_tile[:, cs:512],
            start=True, stop=True, skip_group_check=True)
        e_t = ap.tile([128, 512], MM_DT, tag="et")
        if m >= 0 and qc == 0:
            # full width, mask class m after exp
            e_raw = ap.tile([128, 512], F32, tag="eraw")
            nc.scalar.activation(e_raw[:, :], s_ps[:, :], EXP,
                                 scale=inv_sqrt_d)
            nc.vector.tensor_tensor(
                e_t[:, :], e_raw[:, :], msk_sb[:, m * 512:(m + 1) * 512],
                op=MUL)
        elif m >= 0:
            # trimmed diagonal: triangle block [cs, cs+128) + full tail
            e_raw = ap.tile([128, 128], F32, tag="erawn")
            nc.scalar.activation(e_raw[:, :], s_ps[:, cs:cs + 128], EXP,
                                 scale=inv_sqrt_d)
            nc.vector.tensor_tensor(
                e_t[:, cs:cs + 128], e_raw[:, :], msk_sb[:, 0:128], op=MUL)
            if cs + 128 < 512:
                nc.scalar.activation(
                    e_t[:, cs + 128:512], s_ps[:, cs + 128:512], EXP,
                    scale=inv_sqrt_d)
        else:
            nc.scalar.activation(e_t[:, :], s_ps[:, :], EXP,
                                 scale=inv_sqrt_d)
        nc.tensor.matmul(
            den_ps[:, cs:512], ones_sb[:, :], e_t[:, cs:512],
            start=(i == 0), stop=(i == last), skip_group_check=True)
        nc.tensor.matmul(
            o_ps[:, cs:512],
            vn_sb[kt // 4][:, (kt % 4) * 128:(kt % 4 + 1) * 128],
            e_t[:, cs:512],
            start=(i == 0), stop=(i == last), skip_group_check=True)
    # normalize: broadcast den via K=1 matmul, approx-reciprocal, scale
    den_sb = aop.tile([1, 512], R32, tag="densb")
    nc.scalar.copy(den_sb[:, :], den_ps[:, :])
    bc_ps = mmt("D")
    nc.tensor.matmul(
        bc_ps[:, :], onesr_sb[:, :], den_sb[:, :],
        start=True, stop=True, skip_group_check=True)
    rcp_t = aop.tile([128, 512], F32, tag="rcp")
    nc.vector.reciprocal_approx_fast(rcp_t[:, :], bc_ps[:, :])
    at_t = aop.tile([128, 512], MM_DT, tag="attT")
    nc.vector.tensor_tensor(at_t[:, :], o_ps[:, :], rcp_t[:, :], op=MUL)
    nc.gpsimd.dma_start(attT_loc[qc][h * 128:(h + 1) * 128, :], at_t[:, :])


def _oproj_chunk(nc, qc, mmt, osp, oop, wo_sb, attT_full, out):
    att_r = attT_full[qc].rearrange("(a p) t -> p a t", p=128)
    for f in range(4):
        tt = qc * 4 + f
        strip = osp.tile([128, KT * 128], MM_DT, tag="strip")
        nc.gpsimd.dma_start(
            strip[:, :].rearrange("p (a f) -> p a f", a=KT),
            att_r[:, :, f * 128:(f + 1) * 128])
        o_ps = mmt("AB"[f % 2])
        for k2 in range(KT):
            nc.tensor.matmul(
                o_ps[:, :],
                strip[:, k2 * 128:(k2 + 1) * 128],
                wo_sb[:, k2 * DQ:(k2 + 1) * DQ],
                start=(k2 == 0), stop=(k2 == KT - 1), skip_group_check=True)
        ot = oop.tile([128, 512], F32, tag="ot")
        nc.scalar.copy(ot[:, :], o_ps[:, :])
        nc.gpsimd.dma_start(out[tt * 128:(tt + 1) * 128, :], ot[:, :])


def _host_consts():
    # rope tables, transposed + sign-folded
    inv = 1.0 / (ROPE_BASE ** (np.arange(0, D, 2, dtype=np.float32) / D))
    t = np.arange(T, dtype=np.float32)
    f = np.outer(t, inv)
    e = np.concatenate([f, f], axis=-1)
    cos = np.cos(e).astype(np.float32)
    sin = np.sin(e).astype(np.float32)
    sgn = np.where(np.arange(D) < D // 2, -1.0, 1.0).astype(np.float32)
    cosT = np.ascontiguousarray(cos.T)
    sinT = np.ascontiguousarray((sin * sgn).T)
    # causal 0/1 masks for the 4 diagonal kt-tile classes: keep iff f - p >= 128*m
    p = np.arange(128)[:, None]
    fr = np.arange(512)[None, :]
    msk = np.concatenate(
        [(fr - p >= 128 * m).astype(np.float32) for m in range(4)], axis=1)
    ones = np.ones((128, 1), np.float32)
    onesr = np.ones((1, 128), np.float32)
    ident = np.eye(128, dtype=np.float32)
    return cosT, sinT, msk, ones, onesr, ident


def kernel(x, wq, wk, wv, wo, mask=None, **_ignored):
    BF16 = ml_dtypes.bfloat16
    x = np.asarray(x, dtype=np.float32)
    wq = np.asarray(wq, dtype=np.float32)
    wk = np.asarray(wk, dtype=np.float32)
    wv = np.asarray(wv, dtype=np.float32)
    wo = np.asarray(wo, dtype=np.float32)
    B = x.shape[0]
    xT = np.ascontiguousarray(x.reshape(T, HID).T).astype(BF16)   # [HID, T]
    cosT, sinT, msk, ones, onesr, ident = _host_consts()

    if "nc" not in _BUILD_CACHE:
        _BUILD_CACHE["nc"] = _build_nc()
    nc = _BUILD_CACHE["nc"]

    in_maps = []
    for i in range(NC):
        in_maps.append({
            "xT": xT,
            "wq": np.ascontiguousarray(wq[:, i * DQ:(i + 1) * DQ]).astype(BF16),
            "wk": np.ascontiguousarray(wk[:, i * D:(i + 1) * D]).astype(BF16),
            "wv": np.ascontiguousarray(wv[:, i * D:(i + 1) * D]).astype(BF16),
            "wo": np.ascontiguousarray(wo[:, i * DQ:(i + 1) * DQ]).astype(BF16),
            "cosT": cosT.astype(BF16), "sinT": sinT.astype(BF16),
            "masks": msk.astype(BF16),
            "ones": ones.astype(BF16), "onesr": onesr, "ident": ident,
        })

    res = run_bass_kernel_spmd(nc, in_maps, core_ids=list(range(NC)), **RUN_KWARGS)
    _BUILD_CACHE["last_res"] = res
    out = np.concatenate([res.results[i]["out"] for i in range(NC)], axis=1)
    return out.reshape(B, T, HID)


if __name__ == "__main__":
    rng = np.random.default_rng(0)
    s = 1.0 / math.sqrt(HID)
    x = rng.standard_normal((1, T, HID), dtype=np.float32)
    wq_ = rng.standard_normal((HID, H * D), dtype=np.float32) * s
    wk_ = rng.standard_normal((HID, KV * D), dtype=np.float32) * s
    wv_ = rng.standard_normal((HID, KV * D), dtype=np.float32) * s
    wo_ = rng.standard_normal((H * D, HID), dtype=np.float32) * s
    o = kernel(x, wq_, wk_, wv_, wo_, None)
    print("out", o.shape, o.dtype, float(np.abs(o).mean()))
